# revision 11
# baseline (speedup 1.0000x reference)
"""Trainium2 Bass kernel for nn_Conv3x3 (3x3 stride-3 switched-capacitor conv).

The 18-step charge-integration recurrence in the reference reduces exactly to

    out[i, j] = S * sum_{a,b} w[a, b] * x[3i+a, 3j+b],   S = -C_RATIO*C_BASE/c2

i.e. a plain 3x3 stride-3 correlation scaled by S ~= -1/0.924458, with the
(1536, 1536) patch grid flattened row-major.

Sharding: the 4608-wide second axis of x is split into 8 column slices of 576
(one per NeuronCore); the weight is replicated.  Each core computes a
(1536, 192) column slice of the patch grid; no cross-core communication.

Per-core kernel: the 1536 patch rows map to 12 full 128-partition tiles
(3 consecutive x rows per partition, loaded with one fully contiguous DMA).
The 9 stencil taps are fused multiply-accumulates on the vector engine
(scalar_tensor_tensor), reading stride-3 slices of the row-triple tile.
"""

import math
import sys

import numpy as np

for _p in ("/opt/trn_rl_repo",):
    if _p not in sys.path:
        sys.path.insert(0, _p)

import concourse.bass as bass
import concourse.mybir as mybir
from concourse.tile import TileContext

# ---- problem constants (hardcoded; must match the reference) ----
N_CORES = 8
W = H = 4608
NW, NH = W // 3, H // 3  # 1536, 1536
COLS = H // N_CORES      # 576 input columns per core
NJ = COLS // 3           # 192 output columns per core

C_BASE = 1e-14
C_RATIO = 100 * (2**4 - 1)  # 1500
INIT_C1_SCALED = 0.924458
_C2 = INIT_C1_SCALED * C_BASE * C_RATIO
SCALE = -(C_BASE / _C2) * C_RATIO  # = -1/INIT_C1_SCALED

# tiling knobs
P = 128
KBLOCKS = NW // P            # 12 row-blocks of 128 patch rows
GROUP = 4                    # row-blocks processed per loop iteration
NGROUPS = KBLOCKS // GROUP
BUFS = 3


def _legalize_multiwait(nc: bass.Bass) -> int:
    """Walrus codegen in this toolchain accepts at most ONE sync-wait per
    instruction.  Tile can attach several (e.g. the kernel-tail drain waits on
    every semaphore).  Hoist all but the last wait onto standalone
    EventSemaphore no-ops on the same engine, inserted just before the
    instruction — per-engine program order makes this equivalent."""
    n = 0
    for f in nc.m.functions:
        for bb in f.blocks:
            out = []
            for inst in bb.instructions:
                si = inst.sync_info
                if si is not None and si.on_wait and len(si.on_wait) > 1:
                    waits = list(si.on_wait)
                    for j, w in enumerate(waits[:-1]):
                        ev = mybir.InstEventSemaphore(
                            name=f"{inst.name}-hoistw{j}",
                            opcode="EventSemaphore",
                            engine=inst.engine,
                            ins=[],
                            outs=[],
                            sync_info=mybir.SyncInfo(on_wait=[w], on_update=[]),
                        )
                        try:
                            nc.register_instruction(ev, overwrite=True)
                        except Exception:
                            pass
                        out.append(ev)
                        n += 1
                    si.on_wait = [waits[-1]]
                out.append(inst)
            bb.instructions = out
    return n


def build_nc() -> bass.Bass:
    nc = bass.Bass()
    x = nc.declare_dram_parameter("x", [W, COLS], mybir.dt.float32, isOutput=False)
    wt = nc.declare_dram_parameter("weight", [3, 3], mybir.dt.float32, isOutput=False)
    y = nc.declare_dram_parameter("y", [NW, NJ], mybir.dt.float32, isOutput=True)

    with TileContext(nc) as tc:
        with (
            tc.tile_pool(name="wpool", bufs=1) as wpool,
            tc.tile_pool(name="xpool", bufs=BUFS) as xpool,
            tc.tile_pool(name="ypool", bufs=BUFS) as ypool,
            tc.tile_pool(name="gpool", bufs=6) as gpool,
        ):
            # weight prep: broadcast the 9 taps to all partitions, scale by S
            wraw = wpool.tile([P, 9], mybir.dt.float32)
            nc.sync.dma_start(
                out=wraw[:],
                in_=wt[:].rearrange("a b -> (a b)")[None, :].to_broadcast((P, 9)),
            )
            wbc = wpool.tile([P, 9], mybir.dt.float32)
            nc.vector.tensor_scalar_mul(wbc[:], wraw[:], float(SCALE))

            rows_per_group = 3 * P * GROUP  # x rows consumed per iteration
            for g in range(NGROUPS):
                xt = xpool.tile([P, GROUP * 3 * COLS], mybir.dt.float32)
                src = x[g * rows_per_group:(g + 1) * rows_per_group, :].rearrange(
                    "(g2 p a) c -> p g2 (a c)", g2=GROUP, p=P, a=3
                )
                nc.sync.dma_start(
                    out=xt[:].rearrange("p (g2 r) -> p g2 r", g2=GROUP), in_=src
                )
                # [p][g2][a][b][j] view of the row-triple tile
                xv = xt[:].rearrange(
                    "p (g2 a j b) -> p g2 a b j", g2=GROUP, a=3, j=NJ, b=3
                )
                yt = ypool.tile([P, GROUP * NJ], mybir.dt.float32)
                yv = yt[:].rearrange("p (g2 j) -> p g2 j", g2=GROUP)
                # gate: a trivial op is the first DVE reader of the fresh DMA
                # and carries the (single) DMA wait; walrus allows only one
                # sync wait per compute instruction, and the first real tap
                # would otherwise need two (DMA + engine chain for wbc).
                # Reads only xt so it never needs a second wait itself.
                gate = gpool.tile([P, 1], mybir.dt.float32)
                nc.vector.tensor_scalar_mul(gate[:], xt[:, 0:1], 1.0)
                for k in range(9):
                    a, b = divmod(k, 3)
                    in0 = xv[:, :, a, b, :]
                    if k == 0:
                        nc.vector.tensor_scalar_mul(yv, in0, wbc[:, 0:1])
                    else:
                        nc.vector.scalar_tensor_tensor(
                            out=yv,
                            in0=in0,
                            scalar=wbc[:, k:k + 1],
                            in1=yv,
                            op0=mybir.AluOpType.mult,
                            op1=mybir.AluOpType.add,
                        )
                dst = y[g * GROUP * P:(g + 1) * GROUP * P, :].rearrange(
                    "(g2 p) j -> p g2 j", g2=GROUP, p=P
                )
                nc.sync.dma_start(out=dst, in_=yt[:].rearrange(
                    "p (g2 j) -> p g2 j", g2=GROUP
                ))
    _legalize_multiwait(nc)
    return nc


_CACHED = {}


def _get_nc() -> bass.Bass:
    if "nc" not in _CACHED:
        _CACHED["nc"] = build_nc()
    return _CACHED["nc"]


def kernel(**inputs: np.ndarray) -> np.ndarray:
    from concourse import bass_utils

    x = np.ascontiguousarray(np.asarray(inputs["x"], dtype=np.float32))
    weight = np.ascontiguousarray(np.asarray(inputs["weight"], dtype=np.float32))
    assert x.shape == (W, H) and weight.shape == (3, 3)

    nc = _get_nc()
    in_maps = [
        {
            "x": np.ascontiguousarray(x[:, m * COLS:(m + 1) * COLS]),
            "weight": weight,
        }
        for m in range(N_CORES)
    ]
    res = bass_utils.run_bass_kernel_spmd(nc, in_maps, core_ids=list(range(N_CORES)))
    out2d = np.empty((NW, NH), dtype=np.float32)
    for m in range(N_CORES):
        out2d[:, m * NJ:(m + 1) * NJ] = res.results[m]["y"]
    return out2d.reshape(-1)


# revision 12
# speedup vs baseline: 40422.5385x; 40422.5385x over previous
"""Trainium2 Bass kernel for nn_Conv3x3 (3x3 stride-3 switched-capacitor conv).

The 18-step charge-integration recurrence in the reference reduces exactly to

    out[i, j] = S * sum_{a,b} w[a, b] * x[3i+a, 3j+b],   S = -C_RATIO*C_BASE/c2

i.e. a plain 3x3 stride-3 correlation scaled by S ~= -1/0.924458, with the
(1536, 1536) patch grid flattened row-major.

Sharding: the 4608-wide second axis of x is split into 8 column slices of 576
(one per NeuronCore); the weight is replicated.  Each core computes a
(1536, 192) column slice of the patch grid; no cross-core communication.

Per-core kernel: the 1536 patch rows map to 12 full 128-partition tiles
(3 consecutive x rows per partition, loaded with one fully contiguous DMA).
The 9 stencil taps are fused multiply-accumulates on the vector engine
(scalar_tensor_tensor), reading stride-3 slices of the row-triple tile.
"""

import math
import sys

import numpy as np

for _p in ("/opt/trn_rl_repo",):
    if _p not in sys.path:
        sys.path.insert(0, _p)

import concourse.bass as bass
import concourse.mybir as mybir
from concourse.tile import TileContext

# ---- problem constants (hardcoded; must match the reference) ----
N_CORES = 8
W = H = 4608
NW, NH = W // 3, H // 3  # 1536, 1536
COLS = H // N_CORES      # 576 input columns per core
NJ = COLS // 3           # 192 output columns per core

C_BASE = 1e-14
C_RATIO = 100 * (2**4 - 1)  # 1500
INIT_C1_SCALED = 0.924458
_C2 = INIT_C1_SCALED * C_BASE * C_RATIO
SCALE = -(C_BASE / _C2) * C_RATIO  # = -1/INIT_C1_SCALED

# tiling knobs
P = 128
KBLOCKS = NW // P            # 12 row-blocks of 128 patch rows
GROUP = 4                    # row-blocks processed per loop iteration
NGROUPS = KBLOCKS // GROUP
BUFS = 3


def _legalize_multiwait(nc: bass.Bass) -> int:
    """Walrus codegen in this toolchain accepts at most ONE sync-wait per
    instruction.  Tile can attach several (e.g. the kernel-tail drain waits on
    every semaphore).  Hoist all but the last wait onto standalone
    EventSemaphore no-ops on the same engine, inserted just before the
    instruction — per-engine program order makes this equivalent."""
    n = 0
    for f in nc.m.functions:
        for bb in f.blocks:
            out = []
            for inst in bb.instructions:
                si = inst.sync_info
                if si is not None and si.on_wait and len(si.on_wait) > 1:
                    waits = list(si.on_wait)
                    for j, w in enumerate(waits[:-1]):
                        ev = mybir.InstEventSemaphore(
                            name=f"{inst.name}-hoistw{j}",
                            opcode="EventSemaphore",
                            engine=inst.engine,
                            ins=[],
                            outs=[],
                            sync_info=mybir.SyncInfo(on_wait=[w], on_update=[]),
                        )
                        try:
                            nc.register_instruction(ev, overwrite=True)
                        except Exception:
                            pass
                        out.append(ev)
                        n += 1
                    si.on_wait = [waits[-1]]
                out.append(inst)
            bb.instructions = out
    return n


def build_nc(iters: int = 1) -> bass.Bass:
    """iters > 1 wraps the whole per-core computation in an on-device For_i
    loop (used only for timing; the graded kernel uses iters=1)."""
    nc = bass.Bass()
    x = nc.declare_dram_parameter("x", [W, COLS], mybir.dt.float32, isOutput=False)
    wt = nc.declare_dram_parameter("weight", [3, 3], mybir.dt.float32, isOutput=False)
    y = nc.declare_dram_parameter("y", [NW, NJ], mybir.dt.float32, isOutput=True)

    with TileContext(nc) as tc:
        with (
            tc.tile_pool(name="wpool", bufs=1) as wpool,
            tc.tile_pool(name="xpool", bufs=BUFS) as xpool,
            tc.tile_pool(name="ypool", bufs=BUFS) as ypool,
            tc.tile_pool(name="gpool", bufs=6) as gpool,
        ):
            # weight prep: broadcast the 9 taps to all partitions, scale by S
            wraw = wpool.tile([P, 9], mybir.dt.float32)
            nc.sync.dma_start(
                out=wraw[:],
                in_=wt[:].rearrange("a b -> (a b)")[None, :].to_broadcast((P, 9)),
            )
            wbc = wpool.tile([P, 9], mybir.dt.float32)
            nc.vector.tensor_scalar_mul(wbc[:], wraw[:], float(SCALE))

            def body():
                rows_per_group = 3 * P * GROUP  # x rows consumed per iteration
                for g in range(NGROUPS):
                    xt = xpool.tile([P, GROUP * 3 * COLS], mybir.dt.float32,
                                    name=f"xt{g}", tag="xt")
                    src = x[g * rows_per_group:(g + 1) * rows_per_group, :].rearrange(
                        "(g2 p a) c -> p g2 (a c)", g2=GROUP, p=P, a=3
                    )
                    nc.sync.dma_start(
                        out=xt[:].rearrange("p (g2 r) -> p g2 r", g2=GROUP), in_=src
                    )
                    # [p][g2][a][b][j] view of the row-triple tile
                    xv = xt[:].rearrange(
                        "p (g2 a j b) -> p g2 a b j", g2=GROUP, a=3, j=NJ, b=3
                    )
                    yt = ypool.tile([P, GROUP * NJ], mybir.dt.float32,
                                    name=f"yt{g}", tag="yt")
                    yv = yt[:].rearrange("p (g2 j) -> p g2 j", g2=GROUP)
                    # gate: a trivial op is the first DVE reader of the fresh
                    # DMA and carries the (single) DMA wait; walrus allows only
                    # one sync wait per compute instruction, and the first real
                    # tap would otherwise need two (DMA + engine chain for
                    # wbc).  Reads only xt so it never needs a second wait.
                    gate = gpool.tile([P, 1], mybir.dt.float32,
                                      name=f"gate{g}", tag="gate")
                    nc.vector.tensor_scalar_mul(gate[:], xt[:, 0:1], 1.0)
                    for k in range(9):
                        a, b = divmod(k, 3)
                        in0 = xv[:, :, a, b, :]
                        if k == 0:
                            nc.vector.tensor_scalar_mul(yv, in0, wbc[:, 0:1])
                        else:
                            nc.vector.scalar_tensor_tensor(
                                out=yv,
                                in0=in0,
                                scalar=wbc[:, k:k + 1],
                                in1=yv,
                                op0=mybir.AluOpType.mult,
                                op1=mybir.AluOpType.add,
                            )
                    dst = y[g * GROUP * P:(g + 1) * GROUP * P, :].rearrange(
                        "(g2 p) j -> p g2 j", g2=GROUP, p=P
                    )
                    nc.sync.dma_start(out=dst, in_=yt[:].rearrange(
                        "p (g2 j) -> p g2 j", g2=GROUP
                    ))

            if iters == 1:
                body()
            else:
                with tc.For_i(0, iters, 1):
                    body()
    _legalize_multiwait(nc)
    return nc


_CACHED = {}


def _get_nc() -> bass.Bass:
    if "nc" not in _CACHED:
        _CACHED["nc"] = build_nc()
    return _CACHED["nc"]


def kernel(**inputs: np.ndarray) -> np.ndarray:
    from concourse import bass_utils

    x = np.ascontiguousarray(np.asarray(inputs["x"], dtype=np.float32))
    weight = np.ascontiguousarray(np.asarray(inputs["weight"], dtype=np.float32))
    assert x.shape == (W, H) and weight.shape == (3, 3)

    nc = _get_nc()
    in_maps = [
        {
            "x": np.ascontiguousarray(x[:, m * COLS:(m + 1) * COLS]),
            "weight": weight,
        }
        for m in range(N_CORES)
    ]
    res = bass_utils.run_bass_kernel_spmd(nc, in_maps, core_ids=list(range(N_CORES)))
    out2d = np.empty((NW, NH), dtype=np.float32)
    for m in range(N_CORES):
        out2d[:, m * NJ:(m + 1) * NJ] = res.results[m]["y"]
    return out2d.reshape(-1)
